# revision 25
# baseline (speedup 1.0000x reference)
"""Multi-head GQA attention (RoPE, causal) on 8 TRN2 NeuronCores.

Problem: B=1, S=2048, DIM=2048, 32 Q heads / 8 KV heads, head_dim=64, fp32.

Strategy (tensor parallel over heads, no collectives):
  - Core c owns Q heads 4c..4c+3 and KV head c (GQA group == core).
  - Host pre-transposes x -> xT [D, S] and all weights to [contraction, free]
    layout; RoPE reduced to partition-aligned vector ops by permuting the
    head_dim of wq/wk on the host (even lanes first, odd lanes second).
  - Scores computed transposed (S^T[sk, sq] = K_rot^T_chunk.T @ Q_rot^T) so
    softmax's sum runs over the partition axis, computed for free by a
    ones-row appended to V (row 64 of the AV output = sum(exp)).
  - Single fused pipeline over sq chunks: projections for chunk j+1 and the
    output projection for finished chunks are interleaved between attention
    blocks of chunk j, so the PE never idles long enough for the HAM clock
    gate to re-throttle it to 1.2 GHz (the previous version spent 61% of the
    kernel at half clock) and the ScalarE exp stream overlaps all PE work.
  - exp is batched over both heads of a pair in one ACTIVATE ([128, 2, nw]
    across two PSUM banks) - ScalarE runs ONLY exp; every copy/shuffle is on
    DVE or DMA (cross-partition swaps via SBUF->SBUF DMA, V transpose via the
    DMA xbar).
  - Causal masking: fully-masked blocks skipped; of a diagonal block only the
    first 128 trimmed columns can straddle the diagonal, so the multiplicative
    mask is a single [128, 2, 128] strip.
  - Each core computes a partial x_out_c = attn_c @ woT_c [S, D]; the host
    sums the 8 partials (the "all-reduce after wo").
"""
import sys

if "/opt/trn_rl_repo" not in sys.path:
    sys.path.insert(0, "/opt/trn_rl_repo")

import numpy as np

import concourse.bass as bass
import concourse.tile as tile
from concourse import bacc, mybir
from concourse.bass_utils import run_bass_kernel_spmd

# ---- problem constants (hardcoded per contract) ----
S = 2048          # sequence length
D = 2048          # model dim
NH = 32           # total Q heads
NKV = 8           # total KV heads
DH = 64           # head dim
NCORES = 8
HQ = NH // NCORES     # 4 Q heads per core
SQC = 512             # sq chunk (matmul moving free dim)
SKC = 128             # sk chunk (matmul contraction / partition dim)
DC = 128              # d-chunk for projections
NSQ = S // SQC        # 4
NSK = S // SKC        # 16
NDC = D // DC         # 16

F32 = mybir.dt.float32
BF16 = mybir.dt.bfloat16
EXP = mybir.ActivationFunctionType.Exp

_PROGRAM_CACHE = {}


def build_program(dbg=False):
    """Build the SPMD Bass program (identical on all 8 cores)."""
    key = ("nc", dbg)
    if key in _PROGRAM_CACHE:
        return _PROGRAM_CACHE[key]

    nc = bacc.Bacc("TRN2", target_bir_lowering=False, debug=False,
                   num_devices=NCORES)

    # x pre-blocked on host: x4[p, j, d, jc] = x^T[128 d + p, 512 j + jc]
    x4 = nc.dram_tensor("x4", [128, NSQ, NDC, SQC], BF16,
                        kind="ExternalInput")
    # weights pre-blocked: w2[p, d, o] = w^T[128 d + p, o]
    # wq blocked by head-pair tile first: wq3[p, t, d, o] (o < 128)
    wq3 = nc.dram_tensor("wq3", [128, 2, NDC, 128], BF16,
                         kind="ExternalInput")
    wkv2 = nc.dram_tensor("wkv2", [128, NDC, 2 * DH], BF16,
                          kind="ExternalInput")
    woT = nc.dram_tensor("woT", [HQ * DH, D], BF16, kind="ExternalInput")
    cos4 = nc.dram_tensor("cos4", [128, S], BF16, kind="ExternalInput")
    sin4 = nc.dram_tensor("sin4", [128, S], BF16, kind="ExternalInput")
    maskd = nc.dram_tensor("maskd", [128, 2, SKC], BF16, kind="ExternalInput")
    out = nc.dram_tensor("out", [S, D], BF16, kind="ExternalOutput")
    if dbg:
        krot_d = nc.dram_tensor("krot_d", [128, S], BF16, kind="ExternalOutput")
        qrot_d = nc.dram_tensor("qrot_d", [2, 128, S], BF16,
                                kind="ExternalOutput")
        vaug_d = nc.dram_tensor("vaug_d", [128, NSK, 80], BF16,
                                kind="ExternalOutput")
        attnT_d = nc.dram_tensor("attnT_d", [2, 128, S], BF16,
                                 kind="ExternalOutput")

    with tile.TileContext(nc) as tc:
        with tc.tile_pool(name="const", bufs=1) as cpool, \
             tc.tile_pool(name="persist", bufs=1) as ppool, \
             tc.tile_pool(name="work", bufs=2) as wpool, \
             tc.tile_pool(name="ptp", bufs=4) as ptpool, \
             tc.tile_pool(name="ocp", bufs=6) as ocpool, \
             tc.tile_pool(name="ps", bufs=2, space="PSUM") as psp:

            # ---- constants / weights resident in SBUF ----
            xbig = cpool.tile([128, NSQ, NDC, SQC], BF16, name="xbig")
            wqb = cpool.tile([128, 2, NDC, 128], BF16, name="wqb")
            wkvb = cpool.tile([128, NDC, 2 * DH], BF16, name="wkvb")
            wo_t = cpool.tile([128, 2, D], BF16, name="wo_t")
            cos_t = cpool.tile([128, S], BF16, name="cos_t")
            # sin with baked signs AND pre-swapped 32-row groups: the rope
            # "swap" term is computed by 32-row cross-partition tensor_muls
            # reading (q, sinsw) at the source group and writing the dest
            # group -- no copies/DMAs needed
            sinsw_t = cpool.tile([128, S], BF16, name="sinsw_t")
            mask_t = cpool.tile([128, 2, SKC], BF16, name="mask_t")

            # ---- persistent intermediates ----
            # vaug[:, i, :] = [V[sk chunk i] | 1] used as AV stationary
            # V row pitch padded to 80 elems (160 B) so each dma-transpose dest
            # offset stays 32-byte aligned (xbar requirement)
            vaug = ppool.tile([128, NSK, 80], BF16, name="vaug")
            # Q_rot^T: tile t holds heads (2t, 2t+1) at rows (0:64, 64:128)
            qrot = [ppool.tile([128, S], BF16, name=f"qrot{t}")
                    for t in range(2)]
            # K_rot^T duplicated: rows 0:64 == rows 64:128
            krot = ppool.tile([128, S], BF16, name="krot")
            # attention output transposed, normalized
            attnT = [ppool.tile([128, S], BF16, name=f"attnT{t}")
                     for t in range(2)]

            nc.vector.memset(vaug[:, :, DH:DH + 1], 1.0)

            # ---- DMA prologue: few, large transfers; latency-critical
            # pieces on sync, bulk second-wave loads on the scalar HWDGE
            # queue (idle until the first exp) ----
            # critical-path loads in fine pieces alternated across the two
            # HWDGE queues (~130 GB/s each) so KV/Q unblock ASAP
            nc.sync.dma_start(wkvb[:, 0:8, :], wkv2[:, 0:8, :])
            nc.scalar.dma_start(xbig[:, 0, 4:8, :], x4[:, 0, 4:8, :])
            nc.sync.dma_start(xbig[:, 0, 0:4, :], x4[:, 0, 0:4, :])
            nc.scalar.dma_start(wkvb[:, 8:16, :], wkv2[:, 8:16, :])
            nc.sync.dma_start(xbig[:, 0, 8:12, :], x4[:, 0, 8:12, :])
            nc.scalar.dma_start(xbig[:, 0, 12:16, :], x4[:, 0, 12:16, :])
            nc.scalar.dma_start(wqb[:, 0, :, :], wq3[:, 0, :, :])
            nc.sync.dma_start(cos_t[:], cos4.ap())
            nc.sync.dma_start(sinsw_t[:], sin4.ap())
            nc.sync.dma_start(mask_t[:], maskd.ap())
            nc.scalar.dma_start(wqb[:, 1, :, :], wq3[:, 1, :, :])
            nc.sync.dma_start(xbig[:, 1, 0:8, :], x4[:, 1, 0:8, :])
            nc.scalar.dma_start(xbig[:, 1, 8:16, :], x4[:, 1, 8:16, :])


            def dma_x(j):
                def f():
                    nc.sync.dma_start(xbig[:, j, :, :], x4[:, j, :, :])
                return f

            def dma_wo():
                nc.sync.dma_start(
                    wo_t[:], woT.ap().rearrange("(c p) o -> p c o", p=128))

            # PE warm-up: ~18 dummy matmuls bridge the DMA-bound prologue so
            # the HAM clock gate is at 8/8 when the first projection lands
            zro = cpool.tile([128, SQC], BF16, name="zro")
            nc.vector.memset(zro[:], 0.0)
            for w in range(18):
                wps = psp.tile([128, 2, SQC], F32, name="wps", tag="sts",
                               bufs=2)
                nc.tensor.matmul(wps[:, 0, :], zro[:, 0:128], zro[:],
                                 start=True, stop=True)

            st_kv = {}
            st_q = {}
            st_av = {}

            # ---- pipeline unit emitters ----
            def kv_half(c, half):
                """8 d-chunks of the K|V projection for sq chunk c."""
                def f():
                    if half == 0:
                        st_kv[c] = psp.tile([128, SQC], F32, name=f"kvp{c}",
                                            tag="qps", bufs=2)
                    kvp = st_kv[c]
                    for d in range(8 * half, 8 * half + 8):
                        nc.tensor.matmul(kvp[:], wkvb[:, d, :],
                                         xbig[:, c, d, :],
                                         start=(d == 0), stop=(d == NDC - 1))
                return f

            def rope_k(c):
                """RoPE on K chunk c + V transpose into vaug (DMA xbar)."""
                def f():
                    kvp = st_kv.pop(c)
                    sl = slice(c * SQC, (c + 1) * SQC)
                    kvs = wpool.tile([128, SQC], BF16, name="kvs", tag="kvs",
                                     bufs=4)
                    nc.vector.tensor_copy(kvs[:], kvp[:])
                    t1k = wpool.tile([64, SQC], BF16, name="t1k", tag="t1k",
                                     bufs=2)
                    t2k = wpool.tile([64, SQC], BF16, name="t2k", tag="t2k",
                                     bufs=2)
                    nc.vector.tensor_mul(t1k[:], kvs[0:64, :], cos_t[0:64, sl])
                    for g in range(2):
                        sp = 32 * (g ^ 1)
                        nc.vector.tensor_mul(t2k[32 * g:32 * g + 32, :],
                                             kvs[sp:sp + 32, :],
                                             sinsw_t[sp:sp + 32, sl])
                    nc.vector.tensor_add(krot[0:64, sl], t1k[:], t2k[:])
                    nc.vector.tensor_copy(krot[64:128, sl], krot[0:64, sl])
                    nc.sync.dma_start_transpose(
                        vaug[:, 4 * c:4 * c + 4, 0:DH], kvs[64:128, :])
                return f

            def q_half(j, t, half):
                def f():
                    if half == 0:
                        st_q[(j, t)] = psp.tile([128, SQC], F32,
                                                name=f"qp{j}_{t}",
                                                tag="qps", bufs=2)
                    qp = st_q[(j, t)]
                    for d in range(8 * half, 8 * half + 8):
                        nc.tensor.matmul(qp[:], wqb[:, t, d, :],
                                         xbig[:, j, d, :],
                                         start=(d == 0), stop=(d == NDC - 1))
                return f

            def rope_q(j, t):
                def f():
                    qp = st_q.pop((j, t))
                    sl = slice(j * SQC, (j + 1) * SQC)
                    qs = wpool.tile([128, SQC], BF16, name="qs", tag="qs",
                                    bufs=2)
                    nc.vector.tensor_copy(qs[:], qp[:])
                    t1 = wpool.tile([128, SQC], BF16, name="t1", tag="t1",
                                    bufs=2)
                    t2 = wpool.tile([128, SQC], BF16, name="t2", tag="t2",
                                    bufs=2)
                    nc.vector.tensor_mul(t1[:], qs[:], cos_t[:, sl])
                    for g in range(4):
                        sp = 32 * (g ^ 1)
                        nc.vector.tensor_mul(t2[32 * g:32 * g + 32, :],
                                             qs[sp:sp + 32, :],
                                             sinsw_t[sp:sp + 32, sl])
                    nc.vector.tensor_add(qrot[t][:, sl], t1[:], t2[:])
                return f

            def b_block(j, hp, i):
                """One attention block: scores pair, exp, mask, AV x2."""
                def f():
                    s0 = j * SQC
                    k0 = i * SKC
                    m = i - 4 * j          # diagonal sub-position if >= 0
                    off = 0 if m < 1 else SKC * m
                    if i == 0:
                        st_av[hp] = psp.tile([DH + 1, 2, SQC], F32,
                                             name=f"av{j}_{hp}", tag="av",
                                             bufs=1)
                    av = st_av[hp]
                    st = psp.tile([128, 2, SQC], F32, name="st", tag="sts",
                                  bufs=2)
                    for h in range(2):
                        r0 = 64 * h
                        nc.tensor.matmul(
                            st[:, h, off:SQC], krot[r0:r0 + 64, k0:k0 + SKC],
                            qrot[hp][r0:r0 + 64, s0 + off:s0 + SQC],
                            start=True, stop=True, tile_position=(r0, 0))
                    pt = ptpool.tile([128, 2, SQC], BF16, name="pt", tag="pt",
                                     bufs=8)
                    nc.scalar.activation(pt[:, :, off:SQC], st[:, :, off:SQC],
                                         EXP, scale=0.125)
                    if m >= 0:
                        # only the first 128 trimmed cols straddle the diagonal
                        nc.vector.tensor_mul(pt[:, :, off:off + SKC],
                                             pt[:, :, off:off + SKC],
                                             mask_t[:])
                    for h in range(2):
                        nc.tensor.matmul(av[:, h, off:SQC], vaug[:, i, 0:DH + 1],
                                         pt[:, h, off:SQC],
                                         start=(i == 0), stop=(i == 4 * j + 3))
                return f

            def normalize(j, hp):
                def f():
                    av = st_av.pop(hp)
                    s0 = j * SQC
                    zg = wpool.tile([1, 2, SQC], F32, name="zg", tag="zg",
                                    bufs=2)
                    nc.scalar.copy(zg[:], av[DH:DH + 1, :, :])
                    zr = wpool.tile([1, 2, SQC], F32, name="zr", tag="zr",
                                    bufs=2)
                    nc.vector.reciprocal_approx_fast(zr[:], zg[:])
                    for h in range(2):
                        bc = wpool.tile([64, SQC], F32, name="bc", tag="bc",
                                        bufs=4)
                        nc.gpsimd.partition_broadcast(bc[:], zr[0:1, h, :])
                        nc.vector.tensor_mul(
                            attnT[hp][64 * h:64 * h + 64, s0:s0 + SQC],
                            av[0:DH, h, :], bc[:])
                return f

            st_oc = {}

            def c_quad(si, op):
                """Tail-only: output projection using the freed scores banks
                (2-bank psum tile, 4 matmuls sharing LDWEIGHTS, one evac)."""
                def f():
                    if op == 0:
                        st_oc[si] = ocpool.tile([128, 4, SQC], BF16,
                                                name=f"oc{si}", tag="oc",
                                                bufs=2)
                    oc = st_oc[si]
                    ps2 = psp.tile([128, 2, SQC], F32, name="cps2", tag="sts",
                                   bufs=2)
                    o0 = 2 * op * SQC
                    for t in range(2):
                        for k in range(2):
                            nc.tensor.matmul(
                                ps2[:, k, :],
                                attnT[t][:, si * SKC:(si + 1) * SKC],
                                wo_t[:, t, o0 + k * SQC:o0 + (k + 1) * SQC],
                                start=(t == 0), stop=(t == 1))
                    if op == 0:
                        nc.vector.tensor_copy(oc[:, 0:2, :], ps2[:])
                    else:
                        nc.scalar.copy(oc[:, 2:4, :], ps2[:])
                        st_oc.pop(si)
                        nc.sync.dma_start(
                            out[si * SKC:(si + 1) * SKC, :], oc[:])
                return f

            def c_pair(si, op, tail=False):
                """Output projection for row chunk si, two oi columns."""
                def f():
                    if op == 0:
                        st_oc[si] = ocpool.tile([128, 4, SQC], BF16,
                                                name=f"oc{si}", tag="oc",
                                                bufs=2)
                    oc = st_oc[si]
                    for oi in (2 * op, 2 * op + 1):
                        o0 = oi * SQC
                        ps = psp.tile([128, SQC], F32, name="cps", tag="qps",
                                      bufs=2)
                        for t in range(2):
                            nc.tensor.matmul(
                                ps[:], attnT[t][:, si * SKC:(si + 1) * SKC],
                                wo_t[:, t, o0:o0 + SQC],
                                start=(t == 0), stop=(t == 1))
                        if oi % 2 == 1:
                            nc.scalar.copy(oc[:, oi, :], ps[:])
                        else:
                            nc.vector.tensor_copy(oc[:, oi, :], ps[:])
                    if op == 1:
                        st_oc.pop(si)
                        nc.sync.dma_start(
                            out[si * SKC:(si + 1) * SKC, :], oc[:])
                return f

            # ---- prologue: KV(0), Q(0) ----
            kv_half(0, 0)()
            kv_half(0, 1)()
            rope_k(0)()
            q_half(0, 0, 0)()
            q_half(0, 0, 1)()
            rope_q(0, 0)()
            q_half(0, 1, 0)()
            q_half(0, 1, 1)()
            rope_q(0, 1)()

            # ---- fused pipeline: B(j) blocks with interleaved fillers ----
            def b_units(j):
                units = []
                for hp in range(2):
                    for i in range(4 * j + 4):
                        units.append(b_block(j, hp, i))
                    units.append(normalize(j, hp))
                return units

            fillers = {
                0: [dma_x(2),
                    kv_half(1, 0), kv_half(1, 1), rope_k(1),
                    q_half(1, 0, 0), q_half(1, 0, 1), rope_q(1, 0),
                    q_half(1, 1, 0), q_half(1, 1, 1), rope_q(1, 1)],
                1: [dma_x(3), dma_wo,
                    kv_half(2, 0), kv_half(2, 1), rope_k(2),
                    q_half(2, 0, 0), q_half(2, 0, 1), rope_q(2, 0),
                    q_half(2, 1, 0), q_half(2, 1, 1), rope_q(2, 1)],
                2: [kv_half(3, 0), kv_half(3, 1), rope_k(3),
                    q_half(3, 0, 0), q_half(3, 0, 1), rope_q(3, 0),
                    q_half(3, 1, 0), q_half(3, 1, 1), rope_q(3, 1)]
                   + [c_pair(si, p) for si in (0, 1)
                      for p in (0, 1)],
                3: [c_pair(si, p) for si in (2, 3, 4, 5, 6, 7, 8, 9, 10, 11)
                    for p in (0, 1)],
            }

            LEAD = 3   # emit a few blocks before fillers start pacing so
            #            the exp stream builds a backlog at segment starts
            for j in range(NSQ):
                units = b_units(j)
                fl = fillers[j]
                fi = 0
                n = len(units)
                mfl = len(fl)
                for b, u in enumerate(units):
                    u()
                    if b + 1 <= LEAD:
                        continue
                    tgt = (b + 1 - LEAD) * mfl // (n - LEAD)
                    while fi < tgt:
                        fl[fi]()
                        fi += 1

            # ---- tail: last output-projection chunks ----
            for si in (12, 13, 14, 15):
                for p in (0, 1):
                    c_pair(si, p, tail=True)()

            if dbg:
                nc.sync.dma_start(krot_d.ap(), krot[:])
                nc.sync.dma_start(vaug_d.ap(), vaug[:])
                for t in range(2):
                    nc.sync.dma_start(qrot_d[t, :, :], qrot[t][:])
                    nc.sync.dma_start(attnT_d[t, :, :], attnT[t][:])

    nc.compile()
    _PROGRAM_CACHE[key] = nc
    return nc


def prep_in_maps(x, freqs_cos, freqs_sin, wq, wk, wv, wo):
    """Host-side sharding / pre-transposition. Returns list of 8 in_maps."""
    import ml_dtypes
    bf16 = ml_dtypes.bfloat16

    x = np.asarray(x, dtype=np.float32)
    freqs_cos = np.asarray(freqs_cos, dtype=np.float32)
    freqs_sin = np.asarray(freqs_sin, dtype=np.float32)
    wq = np.asarray(wq, dtype=np.float32)
    wk = np.asarray(wk, dtype=np.float32)
    wv = np.asarray(wv, dtype=np.float32)
    wo = np.asarray(wo, dtype=np.float32)

    xTf = x.reshape(S, D).T                                # [D, S] f32
    # x4[p, j, d, jc] = xT[128 d + p, 512 j + jc]
    x4 = np.ascontiguousarray(
        xTf.reshape(NDC, 128, NSQ, SQC).transpose(1, 2, 0, 3)).astype(bf16)

    # head-dim permutation: even lanes first, odd lanes second
    perm = np.concatenate([np.arange(0, DH, 2), np.arange(1, DH, 2)])
    wq_h = wq.reshape(NH, DH, D)[:, perm, :]               # [NH, DH, D]
    wk_h = wk.reshape(NKV, DH, D)[:, perm, :]              # [NKV, DH, D]
    wv_h = wv.reshape(NKV, DH, D)                          # not permuted

    # cos/sin tiled across the 4 32-row groups: row p -> freq index p % 32
    cosT = np.ascontiguousarray(freqs_cos.T)               # [32, S]
    sinT = np.ascontiguousarray(freqs_sin.T)
    cos4 = np.ascontiguousarray(np.tile(cosT, (4, 1))).astype(bf16)
    # signs baked in AND pre-swapped 32-row groups (the rope swap-mul reads
    # sin at the SOURCE group): out[g] += q[g^1] * sin4[g^1], so row r of
    # sin4 carries the sign of the DEST group r^1: rows 0:32 +sin, 32:64 -sin
    sin4 = np.ascontiguousarray(
        np.tile(np.concatenate([sinT, -sinT], axis=0), (2, 1))).astype(bf16)

    # causal strip mask: within a diagonal block's first 128 trimmed columns,
    # column f passes for partition p iff f >= p (duplicated per head)
    p_idx = np.arange(128)[:, None]
    f_idx = np.arange(SKC)[None, :]
    m2 = (f_idx >= p_idx).astype(bf16)                     # [128, 128]
    maskd = np.ascontiguousarray(
        np.broadcast_to(m2[:, None, :], (128, 2, SKC))).astype(bf16)

    in_maps = []
    for c in range(NCORES):
        wq_c = wq_h[HQ * c:HQ * (c + 1)].reshape(HQ * DH, D)   # [256, D]
        # [128p, 2t, NDC, 128o]
        wq3_c = np.ascontiguousarray(
            wq_c.T.reshape(NDC, 128, 2, 128).transpose(1, 2, 0, 3)
        ).astype(bf16)
        wkv_c = np.concatenate([wk_h[c], wv_h[c]], axis=0)     # [128, D]
        wkv2_c = np.ascontiguousarray(
            wkv_c.T.reshape(NDC, 128, 2 * DH).transpose(1, 0, 2)).astype(bf16)
        woT_c = np.ascontiguousarray(
            wo[:, HQ * DH * c:HQ * DH * (c + 1)].T).astype(bf16)
        in_maps.append({
            "x4": x4, "wq3": wq3_c, "wkv2": wkv2_c, "woT": woT_c,
            "cos4": cos4, "sin4": sin4, "maskd": maskd,
        })
    return in_maps


def run(inputs, trace=False, trace_cores=None, tmpdir=None):
    """Compile (cached), run on 8 cores, gather. Returns (output, results)."""
    nc = build_program()
    in_maps = prep_in_maps(**inputs)
    res = run_bass_kernel_spmd(nc, in_maps, core_ids=list(range(NCORES)),
                               trace=trace, trace_cores=trace_cores,
                               tmpdir=tmpdir)
    acc = np.zeros((S, D), dtype=np.float32)
    for r in res.results:
        acc += r["out"].astype(np.float32)
    return acc.reshape(1, S, D), res


def kernel(**inputs):
    out, _ = run(inputs)
    return out


# revision 26
# speedup vs baseline: 1.1520x; 1.1520x over previous
"""Multi-head GQA attention (RoPE, causal) on 8 TRN2 NeuronCores.

Problem: B=1, S=2048, DIM=2048, 32 Q heads / 8 KV heads, head_dim=64, fp32.

Strategy (tensor parallel over heads, no collectives):
  - Core c owns Q heads 4c..4c+3 and KV head c (GQA group == core).
  - Host pre-transposes x -> xT [D, S] and all weights to [contraction, free]
    layout; RoPE reduced to partition-aligned vector ops by permuting the
    head_dim of wq/wk on the host (even lanes first, odd lanes second).
  - Scores computed transposed (S^T[sk, sq] = K_rot^T_chunk.T @ Q_rot^T) so
    softmax's sum runs over the partition axis, computed for free by a
    ones-row appended to V (row 64 of the AV output = sum(exp)).
  - Single fused pipeline over sq chunks: projections for chunk j+1 and the
    output projection for finished chunks are interleaved between attention
    blocks of chunk j, so the PE never idles long enough for the HAM clock
    gate to re-throttle it to 1.2 GHz (the previous version spent 61% of the
    kernel at half clock) and the ScalarE exp stream overlaps all PE work.
  - exp is batched over both heads of a pair in one ACTIVATE ([128, 2, nw]
    across two PSUM banks) - ScalarE runs ONLY exp; every copy/shuffle is on
    DVE or DMA (cross-partition swaps via SBUF->SBUF DMA, V transpose via the
    DMA xbar).
  - Causal masking: fully-masked blocks skipped; of a diagonal block only the
    first 128 trimmed columns can straddle the diagonal, so the multiplicative
    mask is a single [128, 2, 128] strip.
  - Each core computes a partial x_out_c = attn_c @ woT_c [S, D]; the host
    sums the 8 partials (the "all-reduce after wo").
"""
import sys

if "/opt/trn_rl_repo" not in sys.path:
    sys.path.insert(0, "/opt/trn_rl_repo")

import numpy as np

import concourse.bass as bass
import concourse.tile as tile
from concourse import bacc, mybir
from concourse.bass_utils import run_bass_kernel_spmd

# ---- problem constants (hardcoded per contract) ----
S = 2048          # sequence length
D = 2048          # model dim
NH = 32           # total Q heads
NKV = 8           # total KV heads
DH = 64           # head dim
NCORES = 8
HQ = NH // NCORES     # 4 Q heads per core
SQC = 512             # sq chunk (matmul moving free dim)
SKC = 128             # sk chunk (matmul contraction / partition dim)
DC = 128              # d-chunk for projections
NSQ = S // SQC        # 4
NSK = S // SKC        # 16
NDC = D // DC         # 16

F32 = mybir.dt.float32
BF16 = mybir.dt.bfloat16
EXP = mybir.ActivationFunctionType.Exp

_PROGRAM_CACHE = {}


def build_program(dbg=False):
    """Build the SPMD Bass program (identical on all 8 cores)."""
    key = ("nc", dbg)
    if key in _PROGRAM_CACHE:
        return _PROGRAM_CACHE[key]

    nc = bacc.Bacc("TRN2", target_bir_lowering=False, debug=False,
                   num_devices=NCORES)

    # x pre-blocked on host: x4[p, j, d, jc] = x^T[128 d + p, 512 j + jc]
    x4 = nc.dram_tensor("x4", [128, NSQ, NDC, SQC], BF16,
                        kind="ExternalInput")
    # weights pre-blocked: w2[p, d, o] = w^T[128 d + p, o]
    # wq blocked by head-pair tile first: wq3[p, t, d, o] (o < 128)
    wq3 = nc.dram_tensor("wq3", [128, 2, NDC, 128], BF16,
                         kind="ExternalInput")
    wkv2 = nc.dram_tensor("wkv2", [128, NDC, 2 * DH], BF16,
                          kind="ExternalInput")
    woT = nc.dram_tensor("woT", [HQ * DH, D], BF16, kind="ExternalInput")
    cos4 = nc.dram_tensor("cos4", [128, S], BF16, kind="ExternalInput")
    sin4 = nc.dram_tensor("sin4", [128, S], BF16, kind="ExternalInput")
    maskd = nc.dram_tensor("maskd", [128, 2, SKC], BF16, kind="ExternalInput")
    out = nc.dram_tensor("out", [S, D], BF16, kind="ExternalOutput")
    if dbg:
        krot_d = nc.dram_tensor("krot_d", [128, S], BF16, kind="ExternalOutput")
        qrot_d = nc.dram_tensor("qrot_d", [2, 128, S], BF16,
                                kind="ExternalOutput")
        vaug_d = nc.dram_tensor("vaug_d", [128, NSK, 80], BF16,
                                kind="ExternalOutput")
        attnT_d = nc.dram_tensor("attnT_d", [2, 128, S], BF16,
                                 kind="ExternalOutput")

    with tile.TileContext(nc) as tc:
        with tc.tile_pool(name="const", bufs=1) as cpool, \
             tc.tile_pool(name="persist", bufs=1) as ppool, \
             tc.tile_pool(name="work", bufs=2) as wpool, \
             tc.tile_pool(name="ptp", bufs=4) as ptpool, \
             tc.tile_pool(name="ocp", bufs=6) as ocpool, \
             tc.tile_pool(name="ps", bufs=2, space="PSUM") as psp:

            # ---- constants / weights resident in SBUF ----
            xbig = cpool.tile([128, NSQ, NDC, SQC], BF16, name="xbig")
            wqb = cpool.tile([128, 2, NDC, 128], BF16, name="wqb")
            wkvb = cpool.tile([128, NDC, 2 * DH], BF16, name="wkvb")
            wo_t = cpool.tile([128, 2, D], BF16, name="wo_t")
            cos_t = cpool.tile([128, S], BF16, name="cos_t")
            # sin with baked signs AND pre-swapped 32-row groups: the rope
            # "swap" term is computed by 32-row cross-partition tensor_muls
            # reading (q, sinsw) at the source group and writing the dest
            # group -- no copies/DMAs needed
            sinsw_t = cpool.tile([128, S], BF16, name="sinsw_t")
            mask_t = cpool.tile([128, 2, SKC], BF16, name="mask_t")

            # ---- persistent intermediates ----
            # vaug[:, i, :] = [V[sk chunk i] | 1] used as AV stationary
            # V row pitch padded to 80 elems (160 B) so each dma-transpose dest
            # offset stays 32-byte aligned (xbar requirement)
            vaug = ppool.tile([128, NSK, 80], BF16, name="vaug")
            # Q_rot^T: tile t holds heads (2t, 2t+1) at rows (0:64, 64:128)
            qrot = [ppool.tile([128, S], BF16, name=f"qrot{t}")
                    for t in range(2)]
            # K_rot^T duplicated: rows 0:64 == rows 64:128
            krot = ppool.tile([128, S], BF16, name="krot")
            # attention output transposed, normalized
            attnT = [ppool.tile([128, S], BF16, name=f"attnT{t}")
                     for t in range(2)]

            nc.vector.memset(vaug[:, :, DH:DH + 1], 1.0)

            # ---- DMA prologue: few, large transfers; latency-critical
            # pieces on sync, bulk second-wave loads on the scalar HWDGE
            # queue (idle until the first exp) ----
            # critical-path loads in fine pieces alternated across the two
            # HWDGE queues (~130 GB/s each) so KV/Q unblock ASAP
            nc.sync.dma_start(wkvb[:, 0:8, :], wkv2[:, 0:8, :])
            nc.scalar.dma_start(xbig[:, 0, 4:8, :], x4[:, 0, 4:8, :])
            nc.sync.dma_start(xbig[:, 0, 0:4, :], x4[:, 0, 0:4, :])
            nc.scalar.dma_start(wkvb[:, 8:16, :], wkv2[:, 8:16, :])
            nc.sync.dma_start(xbig[:, 0, 8:12, :], x4[:, 0, 8:12, :])
            nc.scalar.dma_start(xbig[:, 0, 12:16, :], x4[:, 0, 12:16, :])
            nc.scalar.dma_start(wqb[:, 0, :, :], wq3[:, 0, :, :])
            nc.sync.dma_start(cos_t[:], cos4.ap())
            nc.sync.dma_start(sinsw_t[:], sin4.ap())
            nc.sync.dma_start(mask_t[:], maskd.ap())
            nc.scalar.dma_start(wqb[:, 1, :, :], wq3[:, 1, :, :])
            nc.sync.dma_start(xbig[:, 1, 0:8, :], x4[:, 1, 0:8, :])
            nc.scalar.dma_start(xbig[:, 1, 8:16, :], x4[:, 1, 8:16, :])


            def dma_x(j):
                def f():
                    nc.sync.dma_start(xbig[:, j, :, :], x4[:, j, :, :])
                return f

            def dma_wo():
                nc.sync.dma_start(
                    wo_t[:], woT.ap().rearrange("(c p) o -> p c o", p=128))

            # PE warm-up: ~18 dummy matmuls bridge the DMA-bound prologue so
            # the HAM clock gate is at 8/8 when the first projection lands
            zro = cpool.tile([128, SQC], BF16, name="zro")
            nc.vector.memset(zro[:], 0.0)
            for w in range(18):
                wps = psp.tile([128, 2, SQC], F32, name="wps", tag="sts",
                               bufs=2)
                nc.tensor.matmul(wps[:, 0, :], zro[:, 0:128], zro[:],
                                 start=True, stop=True)

            st_kv = {}
            st_q = {}
            st_av = {}

            # ---- pipeline unit emitters ----
            def kv_half(c, half):
                """8 d-chunks of the K|V projection for sq chunk c."""
                def f():
                    if half == 0:
                        st_kv[c] = psp.tile([128, SQC], F32, name=f"kvp{c}",
                                            tag="qps", bufs=2)
                    kvp = st_kv[c]
                    for d in range(8 * half, 8 * half + 8):
                        nc.tensor.matmul(kvp[:], wkvb[:, d, :],
                                         xbig[:, c, d, :],
                                         start=(d == 0), stop=(d == NDC - 1))
                return f

            def rope_k(c):
                """RoPE on K chunk c + V transpose into vaug (DMA xbar)."""
                def f():
                    kvp = st_kv.pop(c)
                    sl = slice(c * SQC, (c + 1) * SQC)
                    kvs = wpool.tile([128, SQC], BF16, name="kvs", tag="kvs",
                                     bufs=4)
                    nc.vector.tensor_copy(kvs[:], kvp[:])
                    t1k = wpool.tile([64, SQC], BF16, name="t1k", tag="t1k",
                                     bufs=2)
                    t2k = wpool.tile([64, SQC], BF16, name="t2k", tag="t2k",
                                     bufs=2)
                    nc.vector.tensor_mul(t1k[:], kvs[0:64, :], cos_t[0:64, sl])
                    for g in range(2):
                        sp = 32 * (g ^ 1)
                        nc.vector.tensor_mul(t2k[32 * g:32 * g + 32, :],
                                             kvs[sp:sp + 32, :],
                                             sinsw_t[sp:sp + 32, sl])
                    nc.vector.tensor_add(krot[0:64, sl], t1k[:], t2k[:])
                    nc.vector.tensor_copy(krot[64:128, sl], krot[0:64, sl])
                    nc.sync.dma_start_transpose(
                        vaug[:, 4 * c:4 * c + 4, 0:DH], kvs[64:128, :])
                return f

            def q_half(j, t, half):
                def f():
                    if half == 0:
                        st_q[(j, t)] = psp.tile([128, SQC], F32,
                                                name=f"qp{j}_{t}",
                                                tag="qps", bufs=2)
                    qp = st_q[(j, t)]
                    for d in range(8 * half, 8 * half + 8):
                        nc.tensor.matmul(qp[:], wqb[:, t, d, :],
                                         xbig[:, j, d, :],
                                         start=(d == 0), stop=(d == NDC - 1))
                return f

            def rope_q(j, t):
                def f():
                    qp = st_q.pop((j, t))
                    sl = slice(j * SQC, (j + 1) * SQC)
                    qs = wpool.tile([128, SQC], BF16, name="qs", tag="qs",
                                    bufs=2)
                    nc.vector.tensor_copy(qs[:], qp[:])
                    t1 = wpool.tile([128, SQC], BF16, name="t1", tag="t1",
                                    bufs=2)
                    t2 = wpool.tile([128, SQC], BF16, name="t2", tag="t2",
                                    bufs=2)
                    nc.vector.tensor_mul(t1[:], qs[:], cos_t[:, sl])
                    for g in range(4):
                        sp = 32 * (g ^ 1)
                        nc.vector.tensor_mul(t2[32 * g:32 * g + 32, :],
                                             qs[sp:sp + 32, :],
                                             sinsw_t[sp:sp + 32, sl])
                    nc.vector.tensor_add(qrot[t][:, sl], t1[:], t2[:])
                return f

            def b_block(j, hp, i):
                """One attention block: scores pair, exp, mask, AV x2."""
                def f():
                    s0 = j * SQC
                    k0 = i * SKC
                    m = i - 4 * j          # diagonal sub-position if >= 0
                    off = 0 if m < 1 else SKC * m
                    if i == 0:
                        st_av[hp] = psp.tile([DH + 1, 2, SQC], F32,
                                             name=f"av{j}_{hp}", tag="av",
                                             bufs=1)
                    av = st_av[hp]
                    st = psp.tile([128, 2, SQC], F32, name="st", tag="sts",
                                  bufs=2)
                    for h in range(2):
                        r0 = 64 * h
                        nc.tensor.matmul(
                            st[:, h, off:SQC], krot[r0:r0 + 64, k0:k0 + SKC],
                            qrot[hp][r0:r0 + 64, s0 + off:s0 + SQC],
                            start=True, stop=True, tile_position=(r0, 0))
                    pt = ptpool.tile([128, 2, SQC], BF16, name="pt", tag="pt",
                                     bufs=8)
                    nc.scalar.activation(pt[:, :, off:SQC], st[:, :, off:SQC],
                                         EXP, scale=0.125)
                    if m >= 0:
                        # only the first 128 trimmed cols straddle the diagonal
                        nc.vector.tensor_mul(pt[:, :, off:off + SKC],
                                             pt[:, :, off:off + SKC],
                                             mask_t[:])
                    for h in range(2):
                        nc.tensor.matmul(av[:, h, off:SQC], vaug[:, i, 0:DH + 1],
                                         pt[:, h, off:SQC],
                                         start=(i == 0), stop=(i == 4 * j + 3))
                return f

            def normalize(j, hp):
                def f():
                    av = st_av.pop(hp)
                    s0 = j * SQC
                    zg = wpool.tile([1, 2, SQC], F32, name="zg", tag="zg",
                                    bufs=2)
                    nc.scalar.copy(zg[:], av[DH:DH + 1, :, :])
                    zr = wpool.tile([1, 2, SQC], F32, name="zr", tag="zr",
                                    bufs=2)
                    nc.vector.reciprocal_approx_fast(zr[:], zg[:])
                    for h in range(2):
                        bc = wpool.tile([64, SQC], F32, name="bc", tag="bc",
                                        bufs=4)
                        nc.gpsimd.partition_broadcast(bc[:], zr[0:1, h, :])
                        nc.vector.tensor_mul(
                            attnT[hp][64 * h:64 * h + 64, s0:s0 + SQC],
                            av[0:DH, h, :], bc[:])
                return f

            st_oc = {}

            def c_pair(si, op, tail=False):
                """Output projection for row chunk si, two oi columns."""
                def f():
                    if op == 0:
                        st_oc[si] = ocpool.tile([128, 4, SQC], BF16,
                                                name=f"oc{si}", tag="oc",
                                                bufs=2)
                    oc = st_oc[si]
                    for oi in (2 * op, 2 * op + 1):
                        o0 = oi * SQC
                        ps = psp.tile([128, SQC], F32, name="cps", tag="qps",
                                      bufs=2)
                        for t in range(2):
                            nc.tensor.matmul(
                                ps[:], attnT[t][:, si * SKC:(si + 1) * SKC],
                                wo_t[:, t, o0:o0 + SQC],
                                start=(t == 0), stop=(t == 1))
                        if oi % 2 == 1:
                            nc.scalar.copy(oc[:, oi, :], ps[:])
                        else:
                            nc.vector.tensor_copy(oc[:, oi, :], ps[:])
                    if op == 1:
                        st_oc.pop(si)
                        nc.sync.dma_start(
                            out[si * SKC:(si + 1) * SKC, :], oc[:])
                return f

            # ---- prologue: KV(0), Q(0) ----
            kv_half(0, 0)()
            kv_half(0, 1)()
            rope_k(0)()
            q_half(0, 0, 0)()
            q_half(0, 0, 1)()
            rope_q(0, 0)()
            q_half(0, 1, 0)()
            q_half(0, 1, 1)()
            rope_q(0, 1)()

            # ---- fused pipeline: B(j) blocks with interleaved fillers ----
            def b_units(j):
                units = []
                for hp in range(2):
                    for i in range(4 * j + 4):
                        units.append(b_block(j, hp, i))
                    units.append(normalize(j, hp))
                return units

            fillers = {
                0: [dma_x(2),
                    kv_half(1, 0), kv_half(1, 1), rope_k(1),
                    q_half(1, 0, 0), q_half(1, 0, 1), rope_q(1, 0),
                    q_half(1, 1, 0), q_half(1, 1, 1), rope_q(1, 1)],
                1: [dma_x(3), dma_wo,
                    kv_half(2, 0), kv_half(2, 1), rope_k(2),
                    q_half(2, 0, 0), q_half(2, 0, 1), rope_q(2, 0),
                    q_half(2, 1, 0), q_half(2, 1, 1), rope_q(2, 1)],
                2: [kv_half(3, 0), kv_half(3, 1), rope_k(3),
                    q_half(3, 0, 0), q_half(3, 0, 1), rope_q(3, 0),
                    q_half(3, 1, 0), q_half(3, 1, 1), rope_q(3, 1)]
                   + [c_pair(si, p) for si in (0, 1)
                      for p in (0, 1)],
                3: [c_pair(si, p) for si in (2, 3, 4, 5, 6, 7, 8, 9, 10, 11)
                    for p in (0, 1)],
            }

            LEAD = 3   # emit a few blocks before fillers start pacing so
            #            the exp stream builds a backlog at segment starts
            for j in range(NSQ):
                units = b_units(j)
                fl = fillers[j]
                fi = 0
                n = len(units)
                mfl = len(fl)
                for b, u in enumerate(units):
                    u()
                    if b + 1 <= LEAD:
                        continue
                    tgt = (b + 1 - LEAD) * mfl // (n - LEAD)
                    while fi < tgt:
                        fl[fi]()
                        fi += 1

            # ---- tail: last output-projection chunks ----
            for si in (12, 13, 14, 15):
                for p in (0, 1):
                    c_pair(si, p, tail=True)()

            if dbg:
                nc.sync.dma_start(krot_d.ap(), krot[:])
                nc.sync.dma_start(vaug_d.ap(), vaug[:])
                for t in range(2):
                    nc.sync.dma_start(qrot_d[t, :, :], qrot[t][:])
                    nc.sync.dma_start(attnT_d[t, :, :], attnT[t][:])

    nc.compile()
    _PROGRAM_CACHE[key] = nc
    return nc


def prep_in_maps(x, freqs_cos, freqs_sin, wq, wk, wv, wo):
    """Host-side sharding / pre-transposition. Returns list of 8 in_maps."""
    import ml_dtypes
    bf16 = ml_dtypes.bfloat16

    x = np.asarray(x, dtype=np.float32)
    freqs_cos = np.asarray(freqs_cos, dtype=np.float32)
    freqs_sin = np.asarray(freqs_sin, dtype=np.float32)
    wq = np.asarray(wq, dtype=np.float32)
    wk = np.asarray(wk, dtype=np.float32)
    wv = np.asarray(wv, dtype=np.float32)
    wo = np.asarray(wo, dtype=np.float32)

    xTf = x.reshape(S, D).T                                # [D, S] f32
    # x4[p, j, d, jc] = xT[128 d + p, 512 j + jc]
    x4 = np.ascontiguousarray(
        xTf.reshape(NDC, 128, NSQ, SQC).transpose(1, 2, 0, 3)).astype(bf16)

    # head-dim permutation: even lanes first, odd lanes second
    perm = np.concatenate([np.arange(0, DH, 2), np.arange(1, DH, 2)])
    wq_h = wq.reshape(NH, DH, D)[:, perm, :]               # [NH, DH, D]
    wk_h = wk.reshape(NKV, DH, D)[:, perm, :]              # [NKV, DH, D]
    wv_h = wv.reshape(NKV, DH, D)                          # not permuted

    # cos/sin tiled across the 4 32-row groups: row p -> freq index p % 32
    cosT = np.ascontiguousarray(freqs_cos.T)               # [32, S]
    sinT = np.ascontiguousarray(freqs_sin.T)
    cos4 = np.ascontiguousarray(np.tile(cosT, (4, 1))).astype(bf16)
    # signs baked in AND pre-swapped 32-row groups (the rope swap-mul reads
    # sin at the SOURCE group): out[g] += q[g^1] * sin4[g^1], so row r of
    # sin4 carries the sign of the DEST group r^1: rows 0:32 +sin, 32:64 -sin
    sin4 = np.ascontiguousarray(
        np.tile(np.concatenate([sinT, -sinT], axis=0), (2, 1))).astype(bf16)

    # causal strip mask: within a diagonal block's first 128 trimmed columns,
    # column f passes for partition p iff f >= p (duplicated per head)
    p_idx = np.arange(128)[:, None]
    f_idx = np.arange(SKC)[None, :]
    m2 = (f_idx >= p_idx).astype(bf16)                     # [128, 128]
    maskd = np.ascontiguousarray(
        np.broadcast_to(m2[:, None, :], (128, 2, SKC))).astype(bf16)

    in_maps = []
    for c in range(NCORES):
        wq_c = wq_h[HQ * c:HQ * (c + 1)].reshape(HQ * DH, D)   # [256, D]
        # [128p, 2t, NDC, 128o]
        wq3_c = np.ascontiguousarray(
            wq_c.T.reshape(NDC, 128, 2, 128).transpose(1, 2, 0, 3)
        ).astype(bf16)
        wkv_c = np.concatenate([wk_h[c], wv_h[c]], axis=0)     # [128, D]
        wkv2_c = np.ascontiguousarray(
            wkv_c.T.reshape(NDC, 128, 2 * DH).transpose(1, 0, 2)).astype(bf16)
        woT_c = np.ascontiguousarray(
            wo[:, HQ * DH * c:HQ * DH * (c + 1)].T).astype(bf16)
        in_maps.append({
            "x4": x4, "wq3": wq3_c, "wkv2": wkv2_c, "woT": woT_c,
            "cos4": cos4, "sin4": sin4, "maskd": maskd,
        })
    return in_maps


def run(inputs, trace=False, trace_cores=None, tmpdir=None):
    """Compile (cached), run on 8 cores, gather. Returns (output, results)."""
    nc = build_program()
    in_maps = prep_in_maps(**inputs)
    res = run_bass_kernel_spmd(nc, in_maps, core_ids=list(range(NCORES)),
                               trace=trace, trace_cores=trace_cores,
                               tmpdir=tmpdir)
    acc = np.zeros((S, D), dtype=np.float32)
    for r in res.results:
        acc += r["out"].astype(np.float32)
    return acc.reshape(1, S, D), res


def kernel(**inputs):
    out, _ = run(inputs)
    return out


# revision 27
# speedup vs baseline: 1.1971x; 1.0391x over previous
"""Multi-head GQA attention (RoPE, causal) on 8 TRN2 NeuronCores.

Problem: B=1, S=2048, DIM=2048, 32 Q heads / 8 KV heads, head_dim=64, fp32.

Strategy (tensor parallel over heads, no collectives):
  - Core c owns Q heads 4c..4c+3 and KV head c (GQA group == core).
  - Host pre-transposes x -> xT [D, S] and all weights to [contraction, free]
    layout; RoPE reduced to partition-aligned vector ops by permuting the
    head_dim of wq/wk on the host (even lanes first, odd lanes second).
  - Scores computed transposed (S^T[sk, sq] = K_rot^T_chunk.T @ Q_rot^T) so
    softmax's sum runs over the partition axis, computed for free by a
    ones-row appended to V (row 64 of the AV output = sum(exp)).
  - Single fused pipeline over sq chunks: projections for chunk j+1 and the
    output projection for finished chunks are interleaved between attention
    blocks of chunk j, so the PE never idles long enough for the HAM clock
    gate to re-throttle it to 1.2 GHz (the previous version spent 61% of the
    kernel at half clock) and the ScalarE exp stream overlaps all PE work.
  - exp is batched over both heads of a pair in one ACTIVATE ([128, 2, nw]
    across two PSUM banks) - ScalarE runs ONLY exp; every copy/shuffle is on
    DVE or DMA (cross-partition swaps via SBUF->SBUF DMA, V transpose via the
    DMA xbar).
  - Causal masking: fully-masked blocks skipped; of a diagonal block only the
    first 128 trimmed columns can straddle the diagonal, so the multiplicative
    mask is a single [128, 2, 128] strip.
  - Each core computes a partial x_out_c = attn_c @ woT_c [S, D]; the host
    sums the 8 partials (the "all-reduce after wo").
"""
import sys

if "/opt/trn_rl_repo" not in sys.path:
    sys.path.insert(0, "/opt/trn_rl_repo")

import numpy as np

import concourse.bass as bass
import concourse.tile as tile
from concourse import bacc, mybir
from concourse.bass_utils import run_bass_kernel_spmd

# ---- problem constants (hardcoded per contract) ----
S = 2048          # sequence length
D = 2048          # model dim
NH = 32           # total Q heads
NKV = 8           # total KV heads
DH = 64           # head dim
NCORES = 8
HQ = NH // NCORES     # 4 Q heads per core
SQC = 512             # sq chunk (matmul moving free dim)
SKC = 128             # sk chunk (matmul contraction / partition dim)
DC = 128              # d-chunk for projections
NSQ = S // SQC        # 4
NSK = S // SKC        # 16
NDC = D // DC         # 16

F32 = mybir.dt.float32
BF16 = mybir.dt.bfloat16
EXP = mybir.ActivationFunctionType.Exp

_PROGRAM_CACHE = {}


def build_program(dbg=False):
    """Build the SPMD Bass program (identical on all 8 cores)."""
    key = ("nc", dbg)
    if key in _PROGRAM_CACHE:
        return _PROGRAM_CACHE[key]

    nc = bacc.Bacc("TRN2", target_bir_lowering=False, debug=False,
                   num_devices=NCORES)

    # x pre-blocked on host: x4[p, j, d, jc] = x^T[128 d + p, 512 j + jc]
    x4 = nc.dram_tensor("x4", [128, NSQ, NDC, SQC], BF16,
                        kind="ExternalInput")
    # weights pre-blocked: w2[p, d, o] = w^T[128 d + p, o]
    # wq blocked by head-pair tile first: wq3[p, t, d, o] (o < 128)
    wq3 = nc.dram_tensor("wq3", [128, 2, NDC, 128], BF16,
                         kind="ExternalInput")
    wkv2 = nc.dram_tensor("wkv2", [128, NDC, 2 * DH], BF16,
                          kind="ExternalInput")
    woT = nc.dram_tensor("woT", [HQ * DH, D], BF16, kind="ExternalInput")
    cos4 = nc.dram_tensor("cos4", [128, S], BF16, kind="ExternalInput")
    sin4 = nc.dram_tensor("sin4", [128, S], BF16, kind="ExternalInput")
    maskd = nc.dram_tensor("maskd", [128, 2, SKC], BF16, kind="ExternalInput")
    out = nc.dram_tensor("out", [S, D], BF16, kind="ExternalOutput")
    if dbg:
        krot_d = nc.dram_tensor("krot_d", [128, S], BF16, kind="ExternalOutput")
        qrot_d = nc.dram_tensor("qrot_d", [2, 128, S], BF16,
                                kind="ExternalOutput")
        vaug_d = nc.dram_tensor("vaug_d", [128, NSK, 80], BF16,
                                kind="ExternalOutput")
        attnT_d = nc.dram_tensor("attnT_d", [2, 128, S], BF16,
                                 kind="ExternalOutput")

    with tile.TileContext(nc) as tc:
        with tc.tile_pool(name="const", bufs=1) as cpool, \
             tc.tile_pool(name="persist", bufs=1) as ppool, \
             tc.tile_pool(name="work", bufs=2) as wpool, \
             tc.tile_pool(name="ptp", bufs=4) as ptpool, \
             tc.tile_pool(name="ocp", bufs=6) as ocpool, \
             tc.tile_pool(name="ps", bufs=2, space="PSUM") as psp:

            # ---- constants / weights resident in SBUF ----
            xbig = cpool.tile([128, NSQ, NDC, SQC], BF16, name="xbig")
            wqb = cpool.tile([128, 2, NDC, 128], BF16, name="wqb")
            wkvb = cpool.tile([128, NDC, 2 * DH], BF16, name="wkvb")
            wo_t = cpool.tile([128, 2, D], BF16, name="wo_t")
            cos_t = cpool.tile([128, S], BF16, name="cos_t")
            # sin with baked signs AND pre-swapped 32-row groups: the rope
            # "swap" term is computed by 32-row cross-partition tensor_muls
            # reading (q, sinsw) at the source group and writing the dest
            # group -- no copies/DMAs needed
            sinsw_t = cpool.tile([128, S], BF16, name="sinsw_t")
            mask_t = cpool.tile([128, 2, SKC], BF16, name="mask_t")

            # ---- persistent intermediates ----
            # vaug[:, i, :] = [V[sk chunk i] | 1] used as AV stationary
            # V row pitch padded to 80 elems (160 B) so each dma-transpose dest
            # offset stays 32-byte aligned (xbar requirement)
            vaug = ppool.tile([128, NSK, 80], BF16, name="vaug")
            # Q_rot^T: tile t holds heads (2t, 2t+1) at rows (0:64, 64:128)
            qrot = [ppool.tile([128, S], BF16, name=f"qrot{t}")
                    for t in range(2)]
            # K_rot^T duplicated: rows 0:64 == rows 64:128
            krot = ppool.tile([128, S], BF16, name="krot")
            # attention output transposed, normalized
            attnT = [ppool.tile([128, S], BF16, name=f"attnT{t}")
                     for t in range(2)]

            nc.vector.memset(vaug[:, :, DH:DH + 1], 1.0)

            # ---- DMA prologue: few, large transfers; latency-critical
            # pieces on sync, bulk second-wave loads on the scalar HWDGE
            # queue (idle until the first exp) ----
            # critical-path loads in fine pieces alternated across the two
            # HWDGE queues (~130 GB/s each) so KV/Q unblock ASAP
            nc.sync.dma_start(wkvb[:, 0:8, :], wkv2[:, 0:8, :])
            nc.scalar.dma_start(xbig[:, 0, 4:8, :], x4[:, 0, 4:8, :])
            nc.sync.dma_start(xbig[:, 0, 0:4, :], x4[:, 0, 0:4, :])
            nc.scalar.dma_start(wkvb[:, 8:16, :], wkv2[:, 8:16, :])
            nc.sync.dma_start(xbig[:, 0, 8:12, :], x4[:, 0, 8:12, :])
            nc.scalar.dma_start(xbig[:, 0, 12:16, :], x4[:, 0, 12:16, :])
            nc.scalar.dma_start(wqb[:, 0, :, :], wq3[:, 0, :, :])
            nc.sync.dma_start(cos_t[:], cos4.ap())
            nc.sync.dma_start(sinsw_t[:], sin4.ap())
            nc.sync.dma_start(mask_t[:], maskd.ap())
            nc.scalar.dma_start(wqb[:, 1, :, :], wq3[:, 1, :, :])
            nc.sync.dma_start(xbig[:, 1, 0:8, :], x4[:, 1, 0:8, :])
            nc.scalar.dma_start(xbig[:, 1, 8:16, :], x4[:, 1, 8:16, :])


            def dma_x(j):
                def f():
                    nc.sync.dma_start(xbig[:, j, :, :], x4[:, j, :, :])
                return f

            def dma_wo():
                nc.sync.dma_start(
                    wo_t[:], woT.ap().rearrange("(c p) o -> p c o", p=128))

            # PE warm-up: ~18 dummy matmuls bridge the DMA-bound prologue so
            # the HAM clock gate is at 8/8 when the first projection lands
            zro = cpool.tile([128, SQC], BF16, name="zro")
            nc.vector.memset(zro[:], 0.0)
            for w in range(18):
                wps = psp.tile([128, 2, SQC], F32, name="wps", tag="sts",
                               bufs=2)
                nc.tensor.matmul(wps[:, 0, :], zro[:, 0:128], zro[:],
                                 start=True, stop=True)

            st_kv = {}
            st_q = {}
            st_av = {}

            # ---- pipeline unit emitters ----
            def kv_half(c, half):
                """8 d-chunks of the K|V projection for sq chunk c."""
                def f():
                    if half == 0:
                        st_kv[c] = psp.tile([128, SQC], F32, name=f"kvp{c}",
                                            tag="qps", bufs=2)
                    kvp = st_kv[c]
                    for d in range(8 * half, 8 * half + 8):
                        nc.tensor.matmul(kvp[:], wkvb[:, d, :],
                                         xbig[:, c, d, :],
                                         start=(d == 0), stop=(d == NDC - 1))
                return f

            def rope_k(c):
                """RoPE on K chunk c + V transpose into vaug (DMA xbar)."""
                def f():
                    kvp = st_kv.pop(c)
                    sl = slice(c * SQC, (c + 1) * SQC)
                    kvs = wpool.tile([128, SQC], BF16, name="kvs", tag="kvs",
                                     bufs=4)
                    nc.vector.tensor_copy(kvs[:], kvp[:])
                    t1k = wpool.tile([64, SQC], BF16, name="t1k", tag="t1k",
                                     bufs=2)
                    t2k = wpool.tile([64, SQC], BF16, name="t2k", tag="t2k",
                                     bufs=2)
                    nc.vector.tensor_mul(t1k[:], kvs[0:64, :], cos_t[0:64, sl])
                    for g in range(2):
                        sp = 32 * (g ^ 1)
                        nc.vector.tensor_mul(t2k[32 * g:32 * g + 32, :],
                                             kvs[sp:sp + 32, :],
                                             sinsw_t[sp:sp + 32, sl])
                    nc.vector.tensor_add(krot[0:64, sl], t1k[:], t2k[:])
                    nc.vector.tensor_copy(krot[64:128, sl], krot[0:64, sl])
                    nc.sync.dma_start_transpose(
                        vaug[:, 4 * c:4 * c + 4, 0:DH], kvs[64:128, :])
                return f

            def q_half(j, t, half):
                def f():
                    if half == 0:
                        st_q[(j, t)] = psp.tile([128, SQC], F32,
                                                name=f"qp{j}_{t}",
                                                tag="qps", bufs=2)
                    qp = st_q[(j, t)]
                    for d in range(8 * half, 8 * half + 8):
                        nc.tensor.matmul(qp[:], wqb[:, t, d, :],
                                         xbig[:, j, d, :],
                                         start=(d == 0), stop=(d == NDC - 1))
                return f

            def rope_q(j, t):
                def f():
                    qp = st_q.pop((j, t))
                    sl = slice(j * SQC, (j + 1) * SQC)
                    qs = wpool.tile([128, SQC], BF16, name="qs", tag="qs",
                                    bufs=2)
                    nc.vector.tensor_copy(qs[:], qp[:])
                    t1 = wpool.tile([128, SQC], BF16, name="t1", tag="t1",
                                    bufs=2)
                    t2 = wpool.tile([128, SQC], BF16, name="t2", tag="t2",
                                    bufs=2)
                    nc.vector.tensor_mul(t1[:], qs[:], cos_t[:, sl])
                    for g in range(4):
                        sp = 32 * (g ^ 1)
                        nc.vector.tensor_mul(t2[32 * g:32 * g + 32, :],
                                             qs[sp:sp + 32, :],
                                             sinsw_t[sp:sp + 32, sl])
                    nc.vector.tensor_add(qrot[t][:, sl], t1[:], t2[:])
                return f

            def b_block(j, hp, i):
                """One attention block: scores pair, exp, mask, AV x2."""
                def f():
                    s0 = j * SQC
                    k0 = i * SKC
                    m = i - 4 * j          # diagonal sub-position if >= 0
                    off = 0 if m < 1 else SKC * m
                    if i == 0:
                        st_av[hp] = psp.tile([DH + 1, 2, SQC], F32,
                                             name=f"av{j}_{hp}", tag="av",
                                             bufs=1)
                    av = st_av[hp]
                    st = psp.tile([128, 2, SQC], F32, name="st", tag="sts",
                                  bufs=2)
                    for h in range(2):
                        r0 = 64 * h
                        nc.tensor.matmul(
                            st[:, h, off:SQC], krot[r0:r0 + 64, k0:k0 + SKC],
                            qrot[hp][r0:r0 + 64, s0 + off:s0 + SQC],
                            start=True, stop=True, tile_position=(r0, 0))
                    pt = ptpool.tile([128, 2, SQC], BF16, name="pt", tag="pt",
                                     bufs=8)
                    nc.scalar.activation(pt[:, :, off:SQC], st[:, :, off:SQC],
                                         EXP, scale=0.125)
                    if m >= 0:
                        # only the first 128 trimmed cols straddle the diagonal
                        nc.vector.tensor_mul(pt[:, :, off:off + SKC],
                                             pt[:, :, off:off + SKC],
                                             mask_t[:])
                    for h in range(2):
                        nc.tensor.matmul(av[:, h, off:SQC], vaug[:, i, 0:DH + 1],
                                         pt[:, h, off:SQC],
                                         start=(i == 0), stop=(i == 4 * j + 3))
                return f

            def normalize(j, hp):
                def f():
                    av = st_av.pop(hp)
                    s0 = j * SQC
                    zg = wpool.tile([1, 2, SQC], F32, name="zg", tag="zg",
                                    bufs=2)
                    nc.scalar.copy(zg[:], av[DH:DH + 1, :, :])
                    zr = wpool.tile([1, 2, SQC], F32, name="zr", tag="zr",
                                    bufs=2)
                    nc.vector.reciprocal_approx_fast(zr[:], zg[:])
                    for h in range(2):
                        bc = wpool.tile([64, SQC], F32, name="bc", tag="bc",
                                        bufs=4)
                        nc.gpsimd.partition_broadcast(bc[:], zr[0:1, h, :])
                        nc.vector.tensor_mul(
                            attnT[hp][64 * h:64 * h + 64, s0:s0 + SQC],
                            av[0:DH, h, :], bc[:])
                return f

            st_oc = {}

            def c_quad(si, op):
                """Tail-only: output projection on the freed scores banks
                (2-bank psum tile, 4 matmuls sharing LDWEIGHTS, one evac)."""
                def f():
                    if op == 0:
                        st_oc[si] = ocpool.tile([128, 4, SQC], BF16,
                                                name=f"oc{si}", tag="oc",
                                                bufs=2)
                    oc = st_oc[si]
                    ps2 = psp.tile([128, 2, SQC], F32, name="cps2", tag="sts",
                                   bufs=2)
                    o0 = 2 * op * SQC
                    for t in range(2):
                        for k in range(2):
                            nc.tensor.matmul(
                                ps2[:, k, :],
                                attnT[t][:, si * SKC:(si + 1) * SKC],
                                wo_t[:, t, o0 + k * SQC:o0 + (k + 1) * SQC],
                                start=(t == 0), stop=(t == 1))
                    if op == 0:
                        nc.vector.tensor_copy(oc[:, 0:2, :], ps2[:])
                    else:
                        nc.scalar.copy(oc[:, 2:4, :], ps2[:])
                        st_oc.pop(si)
                        nc.sync.dma_start(
                            out[si * SKC:(si + 1) * SKC, :], oc[:])
                return f

            def c_pair(si, op, tail=False):
                """Output projection for row chunk si, two oi columns."""
                def f():
                    if op == 0:
                        st_oc[si] = ocpool.tile([128, 4, SQC], BF16,
                                                name=f"oc{si}", tag="oc",
                                                bufs=2)
                    oc = st_oc[si]
                    for oi in (2 * op, 2 * op + 1):
                        o0 = oi * SQC
                        ps = psp.tile([128, SQC], F32, name="cps", tag="qps",
                                      bufs=2)
                        for t in range(2):
                            nc.tensor.matmul(
                                ps[:], attnT[t][:, si * SKC:(si + 1) * SKC],
                                wo_t[:, t, o0:o0 + SQC],
                                start=(t == 0), stop=(t == 1))
                        if oi % 2 == 1:
                            nc.scalar.copy(oc[:, oi, :], ps[:])
                        else:
                            nc.vector.tensor_copy(oc[:, oi, :], ps[:])
                    if op == 1:
                        st_oc.pop(si)
                        nc.sync.dma_start(
                            out[si * SKC:(si + 1) * SKC, :], oc[:])
                return f

            # ---- prologue: KV(0), Q(0) ----
            kv_half(0, 0)()
            kv_half(0, 1)()
            rope_k(0)()
            q_half(0, 0, 0)()
            q_half(0, 0, 1)()
            rope_q(0, 0)()
            q_half(0, 1, 0)()
            q_half(0, 1, 1)()
            rope_q(0, 1)()

            # ---- fused pipeline: B(j) blocks with interleaved fillers ----
            def b_units(j):
                units = []
                for hp in range(2):
                    for i in range(4 * j + 4):
                        units.append(b_block(j, hp, i))
                    units.append(normalize(j, hp))
                return units

            fillers = {
                0: [dma_x(2),
                    kv_half(1, 0), kv_half(1, 1), rope_k(1),
                    q_half(1, 0, 0), q_half(1, 0, 1), rope_q(1, 0),
                    q_half(1, 1, 0), q_half(1, 1, 1), rope_q(1, 1)],
                1: [dma_x(3), dma_wo,
                    kv_half(2, 0), kv_half(2, 1), rope_k(2),
                    q_half(2, 0, 0), q_half(2, 0, 1), rope_q(2, 0),
                    q_half(2, 1, 0), q_half(2, 1, 1), rope_q(2, 1)],
                2: [kv_half(3, 0), kv_half(3, 1), rope_k(3),
                    q_half(3, 0, 0), q_half(3, 0, 1), rope_q(3, 0),
                    q_half(3, 1, 0), q_half(3, 1, 1), rope_q(3, 1)]
                   + [c_pair(si, p) for si in (0, 1)
                      for p in (0, 1)],
                3: [c_pair(si, p) for si in (2, 3, 4, 5, 6, 7, 8, 9, 10, 11)
                    for p in (0, 1)],
            }

            LEAD = 3   # emit a few blocks before fillers start pacing so
            #            the exp stream builds a backlog at segment starts
            for j in range(NSQ):
                units = b_units(j)
                fl = fillers[j]
                fi = 0
                n = len(units)
                mfl = len(fl)
                for b, u in enumerate(units):
                    u()
                    if b + 1 <= LEAD:
                        continue
                    tgt = (b + 1 - LEAD) * mfl // (n - LEAD)
                    while fi < tgt:
                        fl[fi]()
                        fi += 1

            # ---- tail: last output-projection chunks ----
            for si in (12, 13, 14, 15):
                for p in (0, 1):
                    c_quad(si, p)()

            if dbg:
                nc.sync.dma_start(krot_d.ap(), krot[:])
                nc.sync.dma_start(vaug_d.ap(), vaug[:])
                for t in range(2):
                    nc.sync.dma_start(qrot_d[t, :, :], qrot[t][:])
                    nc.sync.dma_start(attnT_d[t, :, :], attnT[t][:])

    nc.compile()
    _PROGRAM_CACHE[key] = nc
    return nc


def prep_in_maps(x, freqs_cos, freqs_sin, wq, wk, wv, wo):
    """Host-side sharding / pre-transposition. Returns list of 8 in_maps."""
    import ml_dtypes
    bf16 = ml_dtypes.bfloat16

    x = np.asarray(x, dtype=np.float32)
    freqs_cos = np.asarray(freqs_cos, dtype=np.float32)
    freqs_sin = np.asarray(freqs_sin, dtype=np.float32)
    wq = np.asarray(wq, dtype=np.float32)
    wk = np.asarray(wk, dtype=np.float32)
    wv = np.asarray(wv, dtype=np.float32)
    wo = np.asarray(wo, dtype=np.float32)

    xTf = x.reshape(S, D).T                                # [D, S] f32
    # x4[p, j, d, jc] = xT[128 d + p, 512 j + jc]
    x4 = np.ascontiguousarray(
        xTf.reshape(NDC, 128, NSQ, SQC).transpose(1, 2, 0, 3)).astype(bf16)

    # head-dim permutation: even lanes first, odd lanes second
    perm = np.concatenate([np.arange(0, DH, 2), np.arange(1, DH, 2)])
    wq_h = wq.reshape(NH, DH, D)[:, perm, :]               # [NH, DH, D]
    wk_h = wk.reshape(NKV, DH, D)[:, perm, :]              # [NKV, DH, D]
    wv_h = wv.reshape(NKV, DH, D)                          # not permuted

    # cos/sin tiled across the 4 32-row groups: row p -> freq index p % 32
    cosT = np.ascontiguousarray(freqs_cos.T)               # [32, S]
    sinT = np.ascontiguousarray(freqs_sin.T)
    cos4 = np.ascontiguousarray(np.tile(cosT, (4, 1))).astype(bf16)
    # signs baked in AND pre-swapped 32-row groups (the rope swap-mul reads
    # sin at the SOURCE group): out[g] += q[g^1] * sin4[g^1], so row r of
    # sin4 carries the sign of the DEST group r^1: rows 0:32 +sin, 32:64 -sin
    sin4 = np.ascontiguousarray(
        np.tile(np.concatenate([sinT, -sinT], axis=0), (2, 1))).astype(bf16)

    # causal strip mask: within a diagonal block's first 128 trimmed columns,
    # column f passes for partition p iff f >= p (duplicated per head)
    p_idx = np.arange(128)[:, None]
    f_idx = np.arange(SKC)[None, :]
    m2 = (f_idx >= p_idx).astype(bf16)                     # [128, 128]
    maskd = np.ascontiguousarray(
        np.broadcast_to(m2[:, None, :], (128, 2, SKC))).astype(bf16)

    in_maps = []
    for c in range(NCORES):
        wq_c = wq_h[HQ * c:HQ * (c + 1)].reshape(HQ * DH, D)   # [256, D]
        # [128p, 2t, NDC, 128o]
        wq3_c = np.ascontiguousarray(
            wq_c.T.reshape(NDC, 128, 2, 128).transpose(1, 2, 0, 3)
        ).astype(bf16)
        wkv_c = np.concatenate([wk_h[c], wv_h[c]], axis=0)     # [128, D]
        wkv2_c = np.ascontiguousarray(
            wkv_c.T.reshape(NDC, 128, 2 * DH).transpose(1, 0, 2)).astype(bf16)
        woT_c = np.ascontiguousarray(
            wo[:, HQ * DH * c:HQ * DH * (c + 1)].T).astype(bf16)
        in_maps.append({
            "x4": x4, "wq3": wq3_c, "wkv2": wkv2_c, "woT": woT_c,
            "cos4": cos4, "sin4": sin4, "maskd": maskd,
        })
    return in_maps


def run(inputs, trace=False, trace_cores=None, tmpdir=None):
    """Compile (cached), run on 8 cores, gather. Returns (output, results)."""
    nc = build_program()
    in_maps = prep_in_maps(**inputs)
    res = run_bass_kernel_spmd(nc, in_maps, core_ids=list(range(NCORES)),
                               trace=trace, trace_cores=trace_cores,
                               tmpdir=tmpdir)
    acc = np.zeros((S, D), dtype=np.float32)
    for r in res.results:
        acc += r["out"].astype(np.float32)
    return acc.reshape(1, S, D), res


def kernel(**inputs):
    out, _ = run(inputs)
    return out
